# revision 1
# baseline (speedup 1.0000x reference)
"""Causal dot-product attention (low-rank V) on 8 Trainium2 NeuronCores.

Problem: inputs [B=4, N=4096, E=1024], Wq/Wk/Wvdown [E, D=256], Wvup [D, E].
    Q = x Wq; K = x Wk; S = Q K^T / sqrt(D) (causal); A = softmax(S)
    V = x Wvdown Wvup; out = A V

Sharding: core = (batch, key-parity). Each of the 4 batches is handled by a
pair of cores; core parity c owns the interleaved global key blocks {2j+c}
(128 rows each), which balances the causal work exactly. Each core computes
full Q for its batch, K/V for its key half, and produces the *unnormalized*
attention output O_unnorm[4096, 1024] plus softmax row-sums. The host
combines: out = (O_even + O_odd) / (s_even + s_odd).

In-kernel layout: scores are computed transposed, ST[k, q] = K Q^T, so that
(a) softmax sums over k are a ones-vector matmul, (b) the exp'd tile P[k, q]
is directly the stationary operand of the O = P^T V matmul (no transposes
on-device at all; the host pre-transposes the activations once).
"""

import sys

sys.path.insert(0, "/opt/trn_rl_repo")

import numpy as np

import concourse.bacc as bacc
import concourse.mybir as mybir
import concourse.tile as tile

F32 = mybir.dt.float32
F32R = mybir.dt.float32r
BF16 = mybir.dt.bfloat16

B, N, E, D = 4, 4096, 1024, 256
NCORES = 8
KLOC = N // 2  # local keys per core
NKB = KLOC // 128  # 16 local key blocks
NQC = N // 512  # 8 query chunks of 512
NKC = KLOC // 512  # 4 local key chunks of 512
SCALE = 1.0 / np.sqrt(np.float32(D))  # 1/16

_CACHE = {}


def _r(ap):
    """View an fp32 AP as float32r so the PE runs at full (1 cyc/row) rate."""
    return ap.bitcast(F32R)


def _build_nc(reps=1):
    nc = bacc.Bacc("TRN2", target_bir_lowering=False)

    xT = nc.dram_tensor("xT", [E, N], F32R, kind="ExternalInput")
    xkT = nc.dram_tensor("xkT", [E, KLOC], F32R, kind="ExternalInput")
    wq = nc.dram_tensor("wq", [E, D], F32R, kind="ExternalInput")
    wk = nc.dram_tensor("wk", [E, D], F32R, kind="ExternalInput")
    wvd = nc.dram_tensor("wvd", [E, D], F32R, kind="ExternalInput")
    wvu = nc.dram_tensor("wvu", [D, E], F32R, kind="ExternalInput")
    mka = nc.dram_tensor("mka", [128, 512], BF16, kind="ExternalInput")
    mkb = nc.dram_tensor("mkb", [128, 512], BF16, kind="ExternalInput")

    o = nc.dram_tensor("o", [N, E], F32, kind="ExternalOutput")
    ssum = nc.dram_tensor("ssum", [NQC, 512], F32, kind="ExternalOutput")

    with tile.TileContext(nc) as tc:
      for _rep in range(reps):
        with (
            tc.tile_pool(name=f"res{_rep}", bufs=1) as res,
            tc.tile_pool(name=f"consts{_rep}", bufs=1) as consts,
        ):
            # Resident results of the projection phase.
            qt = [res.tile([128, N], F32R, tag=f"qt{d}", name=f"qt{d}") for d in range(2)]
            kt = [res.tile([128, KLOC], F32R, tag=f"kt{d}", name=f"kt{d}") for d in range(2)]
            vt = [res.tile([128, E], BF16, tag=f"v{kb}", name=f"v{kb}") for kb in range(NKB)]

            ones_f = consts.tile([128, 1], F32, tag="ones_f")
            nc.vector.memset(ones_f, 1.0)
            ones = consts.tile([128, 1], BF16, tag="ones")
            nc.vector.tensor_copy(ones, ones_f)
            mask_a = consts.tile([128, 512], BF16, tag="mka")
            mask_b = consts.tile([128, 512], BF16, tag="mkb")

            # ---------------- projections ----------------
            with (
                tc.tile_pool(name="wpool", bufs=1) as wp,
                tc.tile_pool(name="xstream", bufs=2) as xs,
                tc.tile_pool(name="vdtp", bufs=1) as vdp,
                tc.tile_pool(name="pproj", bufs=4, space="PSUM") as pp,
            ):
                wq_t = [wp.tile([128, D], F32R, tag=f"wq{c}", name=f"wq{c}") for c in range(8)]
                wk_t = [wp.tile([128, D], F32R, tag=f"wk{c}", name=f"wk{c}") for c in range(8)]
                wvd_t = [wp.tile([128, D], F32R, tag=f"wvd{c}", name=f"wvd{c}") for c in range(8)]
                wvu_t = [wp.tile([128, E], F32R, tag=f"wvu{d}", name=f"wvu{d}") for d in range(2)]
                for c in range(8):
                    sl = slice(c * 128, (c + 1) * 128)
                    nc.gpsimd.dma_start(out=wk_t[c], in_=wk[sl, :])
                    nc.gpsimd.dma_start(out=wvd_t[c], in_=wvd[sl, :])
                for c in range(8):
                    sl = slice(c * 128, (c + 1) * 128)
                    nc.gpsimd.dma_start(out=wq_t[c], in_=wq[sl, :])
                for d in range(2):
                    nc.gpsimd.dma_start(
                        out=wvu_t[d], in_=wvu[d * 128 : (d + 1) * 128, :]
                    )
                nc.gpsimd.dma_start(out=mask_a, in_=mka[:, :])
                nc.gpsimd.dma_start(out=mask_b, in_=mkb[:, :])

                vdt = [vdp.tile([128, KLOC], F32R, tag=f"vdt{d}", name=f"vdt{d}") for d in range(2)]

                # Merged streaming loop: iteration i does KT/VdT for key chunk
                # kc=i (first 4 iterations), QT for query chunk qc=i, and V for
                # key blocks 2i, 2i+1. Each 2MB x-tile is loaded as two halves
                # split across the two HWDGE queues (sync + scalar) so DMA
                # stays ahead of the PE everywhere.
                for i in range(NQC):
                    if i < NKC:
                        xk_h = []
                        for h, eng in ((0, nc.scalar), (1, nc.sync)):
                            xkh = xs.tile(
                                [128, 4, 512], F32R, tag="xk", bufs=2, name=f"xk{h}"
                            )
                            eng.dma_start(
                                out=xkh,
                                in_=xkT[
                                    h * 512 : (h + 1) * 512, i * 512 : (i + 1) * 512
                                ].rearrange("(c p) q -> p c q", p=128),
                            )
                            xk_h.append(xkh)
                    xq_h = []
                    for h, eng in ((0, nc.sync), (1, nc.scalar)):
                        xqh = xs.tile(
                            [128, 4, 512], F32R, tag="xq", bufs=3, name=f"xq{h}"
                        )
                        eng.dma_start(
                            out=xqh,
                            in_=xT[
                                h * 512 : (h + 1) * 512, i * 512 : (i + 1) * 512
                            ].rearrange("(c p) q -> p c q", p=128),
                        )
                        xq_h.append(xqh)

                    if i < NKC:
                        for w_t, dst in ((wk_t, kt), (wvd_t, vdt)):
                            for d in range(2):
                                ps = pp.tile([128, 512], F32, tag="ps")
                                dsl = slice(d * 128, (d + 1) * 128)
                                for c in range(8):
                                    nc.tensor.matmul(
                                        ps,
                                        lhsT=(w_t[c][:, dsl]),
                                        rhs=(xk_h[c // 4][:, c % 4, :]),
                                        start=(c == 0),
                                        stop=(c == 7),
                                    )
                                nc.vector.tensor_copy(
                                    dst[d][:, i * 512 : (i + 1) * 512], ps
                                )

                    for d in range(2):
                        ps = pp.tile([128, 512], F32, tag="ps")
                        dsl = slice(d * 128, (d + 1) * 128)
                        for c in range(8):
                            nc.tensor.matmul(
                                ps,
                                lhsT=(wq_t[c][:, dsl]),
                                rhs=(xq_h[c // 4][:, c % 4, :]),
                                start=(c == 0),
                                stop=(c == 7),
                            )
                        nc.vector.tensor_copy(qt[d][:, i * 512 : (i + 1) * 512], ps)

                    for kb in (2 * i, 2 * i + 1):
                        ksl = slice(kb * 128, (kb + 1) * 128)
                        for eh in range(2):
                            ps = pp.tile([128, 512], F32, tag="ps")
                            esl = slice(eh * 512, (eh + 1) * 512)
                            for d in range(2):
                                nc.tensor.matmul(
                                    ps,
                                    lhsT=(vdt[d][:, ksl]),
                                    rhs=(wvu_t[d][:, esl]),
                                    start=(d == 0),
                                    stop=(d == 1),
                                )
                            nc.vector.tensor_copy(vt[kb][:, esl], ps)

            # ---------------- attention ----------------
            with (
                tc.tile_pool(name="ppool", bufs=1) as ppool,
                tc.tile_pool(name="stage", bufs=3) as stage,
                tc.tile_pool(name="ps_sc", bufs=2, space="PSUM") as ps_sc,
                tc.tile_pool(name="ps_sum", bufs=2, space="PSUM") as ps_sum,
                tc.tile_pool(name="ps_o", bufs=3, space="PSUM") as ps_o,
            ):
                for qc in range(NQC):
                    nb = 2 * qc + 2  # local key blocks this query chunk attends to
                    qsl = slice(qc * 512, (qc + 1) * 512)
                    pts = []
                    for kb in range(nb):
                        ksl = slice(kb * 128, (kb + 1) * 128)
                        st = ps_sc.tile([128, 512], F32, tag="st")
                        for d in range(2):
                            nc.tensor.matmul(
                                st,
                                lhsT=(kt[d][:, ksl]),
                                rhs=(qt[d][:, qsl]),
                                start=(d == 0),
                                stop=(d == 1),
                            )
                        pt = ppool.tile([128, 512], BF16, tag=f"p{kb}", name=f"p{kb}")
                        nc.scalar.activation(
                            pt, st, mybir.ActivationFunctionType.Exp, scale=float(SCALE)
                        )
                        # The last two blocks straddle the causal diagonal.
                        if kb == nb - 2:
                            nc.vector.tensor_mul(pt, pt, mask_a)
                        elif kb == nb - 1:
                            nc.vector.tensor_mul(pt, pt, mask_b)
                        pts.append(pt)

                    # softmax denominators: sums[1, q] += 1^T P[k, q]
                    sums = ps_sum.tile([1, 512], F32, tag="sums")
                    for kb in range(nb):
                        nc.tensor.matmul(
                            sums,
                            lhsT=(ones),
                            rhs=(pts[kb]),
                            start=(kb == 0),
                            stop=(kb == nb - 1),
                        )
                    ssb = stage.tile([1, 512], F32, tag="ssb")
                    nc.vector.tensor_copy(ssb, sums)
                    nc.sync.dma_start(out=ssum[qc : qc + 1, :], in_=ssb)

                    # O[q, e] += P[k, q]^T V[k, e]
                    for q4 in range(4):
                        qb = qc * 4 + q4
                        q4sl = slice(q4 * 128, (q4 + 1) * 128)
                        for eh in range(2):
                            esl = slice(eh * 512, (eh + 1) * 512)
                            ops = ps_o.tile([128, 512], F32, tag="ops")
                            for kb in range(nb):
                                nc.tensor.matmul(
                                    ops,
                                    lhsT=(pts[kb][:, q4sl]),
                                    rhs=(vt[kb][:, esl]),
                                    start=(kb == 0),
                                    stop=(kb == nb - 1),
                                )
                            ob = stage.tile([128, 512], F32, tag="ob")
                            nc.vector.tensor_copy(ob, ops)
                            nc.sync.dma_start(
                                out=o[qb * 128 : (qb + 1) * 128, esl], in_=ob
                            )
    nc.finalize()
    return nc


def _get_nc():
    if "nc" not in _CACHE:
        _CACHE["nc"] = _build_nc()
    return _CACHE["nc"]


def _host_masks(parity: int):
    y = np.arange(512)[None, :]
    x = np.arange(128)[:, None]
    import ml_dtypes

    mask_a = (y - x - 128 * parity >= 0).astype(ml_dtypes.bfloat16)
    mask_b = (y - x - 256 - 128 * parity >= 0).astype(ml_dtypes.bfloat16)
    return mask_a, mask_b


def kernel(inputs, Wq, Wk, Wvdown, Wvup):
    from concourse.bass_utils import run_bass_kernel_spmd

    inputs = np.asarray(inputs, dtype=np.float32)
    Wq = np.ascontiguousarray(np.asarray(Wq, dtype=np.float32))
    Wk = np.ascontiguousarray(np.asarray(Wk, dtype=np.float32))
    Wvdown = np.ascontiguousarray(np.asarray(Wvdown, dtype=np.float32))
    Wvup = np.ascontiguousarray(np.asarray(Wvup, dtype=np.float32))

    nc = _get_nc()

    in_maps = []
    for core in range(NCORES):
        b, parity = core // 2, core % 2
        xb = inputs[b]  # [N, E]
        xT = np.ascontiguousarray(xb.T)  # [E, N]
        xk = np.ascontiguousarray(
            xb.reshape(N // 128, 128, E)[parity::2].reshape(KLOC, E)
        )
        xkT = np.ascontiguousarray(xk.T)  # [E, KLOC]
        mask_a, mask_b = _host_masks(parity)
        in_maps.append(
            {
                "xT": xT,
                "xkT": xkT,
                "wq": Wq,
                "wk": Wk,
                "wvd": Wvdown,
                "wvu": Wvup,
                "mka": mask_a,
                "mkb": mask_b,
            }
        )

    res = run_bass_kernel_spmd(nc, in_maps, core_ids=list(range(NCORES)))
    results = res.results

    out = np.empty((B, N, E), dtype=np.float32)
    for b in range(B):
        o_sum = results[2 * b]["o"] + results[2 * b + 1]["o"]
        s_sum = (results[2 * b]["ssum"] + results[2 * b + 1]["ssum"]).reshape(N)
        out[b] = o_sum / s_sum[:, None]
    return out



# revision 6
# speedup vs baseline: 1.3237x; 1.3237x over previous
"""Causal dot-product attention (low-rank V) on 8 Trainium2 NeuronCores.

Problem: inputs [B=4, N=4096, E=1024], Wq/Wk/Wvdown [E, D=256], Wvup [D, E].
    Q = x Wq; K = x Wk; S = Q K^T / sqrt(D) (causal); A = softmax(S)
    V = x Wvdown Wvup; out = A V

Sharding: core = (batch, key-parity). Each of the 4 batches is handled by a
pair of cores; core parity c owns the interleaved global key blocks {2j+c}
(128 rows each), which balances the causal work exactly. Each core produces
the *unnormalized* partial output plus softmax row-sums; the host combines
out = (O_even + O_odd) / (s_even + s_odd).

v2 key ideas vs the v1 kernel:
  * Low-rank reorder: out = (P^T (x Wvd)) Wvu instead of P^T (x Wvd Wvu).
    The attention contraction runs against rank-256 Vd, then one small
    [q,256]x[256,E] matmul per query chunk. This cuts PE work ~40%: every
    matmul pays an exposed LDWEIGHTS (~1cyc/col bf16, 2cyc/col fp32r) plus
    free-dim stream cycles, and the old O-phase paid 576 P-block reloads.
  * All stationary operands are bf16 (Wq/Wk blocks, x-blocks for Vd, K^T,
    Vd, O'^T), halving weight-load time; moving operands stay fp32r
    (1 cyc/row at free>=256) where precision matters (x, Q, Wvu).
  * Projection iteration i is interleaved with attention on query chunk i:
    qc=i needs exactly key blocks 0..2i+1, which are done by iteration i.
    This keeps the PE queue dense from the first microsecond.
  * Softmax denominators: P tiles vector-added in groups of 4, then a short
    ones-matmul chain (20 matmuls instead of 72).
  * Output partials in bf16 (halves output DMA; host combines in f32).
"""

import sys

sys.path.insert(0, "/opt/trn_rl_repo")

import numpy as np

import concourse.bacc as bacc
import concourse.mybir as mybir
import concourse.tile as tile

F32 = mybir.dt.float32
F32R = mybir.dt.float32r
BF16 = mybir.dt.bfloat16

B, N, E, D = 4, 4096, 1024, 256
NCORES = 8
KLOC = N // 2  # local keys per core
NKB = KLOC // 128  # 16 local key blocks
NQC = N // 512  # 8 query chunks of 512
NKC = KLOC // 512  # 4 local key chunks of 512
SCALE = 1.0 / np.sqrt(np.float32(D))  # 1/16

_CACHE = {}


def _r(ap):
    """View an fp32 AP as float32r so the PE runs at full (1 cyc/row) rate."""
    return ap.bitcast(F32R)


def _build_nc(reps=1):
    nc = bacc.Bacc("TRN2", target_bir_lowering=False)

    xT = nc.dram_tensor("xT", [E, N], F32R, kind="ExternalInput")
    xkT = nc.dram_tensor("xkT", [E, KLOC], F32R, kind="ExternalInput")
    wq = nc.dram_tensor("wq", [E, D], F32, kind="ExternalInput")
    wk = nc.dram_tensor("wk", [E, D], F32, kind="ExternalInput")
    wvd = nc.dram_tensor("wvd", [E, D], F32, kind="ExternalInput")
    wvu = nc.dram_tensor("wvu", [D, E], F32, kind="ExternalInput")
    mka = nc.dram_tensor("mka", [128, 512], BF16, kind="ExternalInput")
    mkb = nc.dram_tensor("mkb", [128, 512], BF16, kind="ExternalInput")

    o = nc.dram_tensor("o", [N, E], BF16, kind="ExternalOutput")
    ssum = nc.dram_tensor("ssum", [NQC, 512], F32, kind="ExternalOutput")

    with tile.TileContext(nc) as tc:
      for _rep in range(reps):
        with (
            tc.tile_pool(name=f"res{_rep}", bufs=1) as res,
            tc.tile_pool(name=f"consts{_rep}", bufs=1) as consts,
            tc.tile_pool(name=f"wpool{_rep}", bufs=1) as wp,
            tc.tile_pool(name=f"wstage{_rep}", bufs=2) as ws,
            tc.tile_pool(name=f"xstream{_rep}", bufs=2) as xs,
            tc.tile_pool(name=f"ppool{_rep}", bufs=1) as ppool,
            tc.tile_pool(name=f"stage{_rep}", bufs=3) as stage,
            tc.tile_pool(name=f"psA{_rep}", bufs=3, space="PSUM") as psA,
            tc.tile_pool(name=f"psB{_rep}", bufs=2, space="PSUM") as psB,
            tc.tile_pool(name=f"psOT{_rep}", bufs=1, space="PSUM") as psOT,
            tc.tile_pool(name=f"psS{_rep}", bufs=1, space="PSUM") as psS,
        ):
            # ---- residents ----
            qt = [res.tile([128, N], BF16, tag=f"qt{d}", name=f"qt{d}") for d in range(2)]
            kt = [res.tile([128, KLOC], BF16, tag=f"kt{d}", name=f"kt{d}") for d in range(2)]
            vd = [res.tile([128, D], BF16, tag=f"vd{kb}", name=f"vd{kb}") for kb in range(NKB)]

            ones_f = consts.tile([128, 1], F32, tag="ones_f")
            nc.vector.memset(ones_f, 1.0)
            ones = consts.tile([128, 1], BF16, tag="ones")
            nc.vector.tensor_copy(ones, ones_f)
            mask_a = consts.tile([128, 512], BF16, tag="mka")
            mask_b = consts.tile([128, 512], BF16, tag="mkb")

            # ---- weights: DMA (gpsimd queue); Wq/Wk cast to bf16 ----
            wq_b = [wp.tile([128, D], BF16, tag=f"wqb{c}", name=f"wqb{c}") for c in range(8)]
            wk_b = [wp.tile([128, D], BF16, tag=f"wkb{c}", name=f"wkb{c}") for c in range(8)]
            wvd_b = [wp.tile([128, D], BF16, tag=f"wvdb{c}", name=f"wvdb{c}") for c in range(8)]
            wvu_b = [wp.tile([128, E], BF16, tag=f"wvub{d}", name=f"wvub{d}") for d in range(2)]
            for c in range(8):
                sl = slice(c * 128, (c + 1) * 128)
                wtmp = ws.tile([128, 3, D], F32, tag="wtmp", name=f"wqk{c}")
                nc.gpsimd.dma_start(out=wtmp[:, 0, :], in_=wq[sl, :])
                nc.gpsimd.dma_start(out=wtmp[:, 1, :], in_=wk[sl, :])
                nc.gpsimd.dma_start(out=wtmp[:, 2, :], in_=wvd[sl, :])
                nc.vector.tensor_copy(wq_b[c], wtmp[:, 0, :])
                nc.vector.tensor_copy(wk_b[c], wtmp[:, 1, :])
                nc.vector.tensor_copy(wvd_b[c], wtmp[:, 2, :])
            for d in range(2):
                wtm2 = ws.tile([128, E], F32, tag="wtm2", name=f"wvuf{d}")
                nc.gpsimd.dma_start(out=wtm2, in_=wvu[d * 128 : (d + 1) * 128, :])
                nc.vector.tensor_copy(wvu_b[d], wtm2)
            nc.gpsimd.dma_start(out=mask_a, in_=mka[:, :])
            nc.gpsimd.dma_start(out=mask_b, in_=mkb[:, :])

            # ---- merged loop: projections(i) + attention(qc=i) ----
            for i in range(NQC):
                qsl = slice(i * 512, (i + 1) * 512)
                # x^T query chunk i, two 512-row halves on the two HWDGE queues
                xq_h = []
                for h, eng in ((0, nc.sync), (1, nc.scalar)):
                    xqh = xs.tile([128, 4, 512], F32R, tag=f"xq{h}", bufs=2, name=f"xq{h}_{i}")
                    eng.dma_start(
                        out=xqh,
                        in_=xT[
                            h * 512 : (h + 1) * 512, qsl
                        ].rearrange("(c p) q -> p c q", p=128),
                    )
                    xq_h.append(xqh)
                xqb_h = []
                for h in range(2):
                    xqb = xs.tile([128, 4, 512], BF16, tag=f"xqb{h}", bufs=2, name=f"xqb{h}_{i}")
                    nc.vector.tensor_copy(xqb, xq_h[h].bitcast(F32))
                    xqb_h.append(xqb)
                if i < NKC:
                    xk_h = []
                    for h, eng in ((0, nc.scalar), (1, nc.sync)):
                        xkh = xs.tile([128, 4, 512], F32R, tag=f"xk{h}", bufs=2, name=f"xk{h}_{i}")
                        eng.dma_start(
                            out=xkh,
                            in_=xkT[
                                h * 512 : (h + 1) * 512, i * 512 : (i + 1) * 512
                            ].rearrange("(c p) q -> p c q", p=128),
                        )
                        xk_h.append(xkh)
                    # bf16 copy of the key chunk: stationary blocks for Vd
                    xkb_h = []
                    for h in range(2):
                        xkb = xs.tile([128, 4, 512], BF16, tag=f"xkb{h}", bufs=2, name=f"xkb{h}_{i}")
                        nc.vector.tensor_copy(xkb, xk_h[h].bitcast(F32))
                        xkb_h.append(xkb)

                # QT: qt[d][:, qsl] = (x Wq)^T   (bf16 stationary, f32r moving)
                for d in range(2):
                    dsl = slice(d * 128, (d + 1) * 128)
                    ps = psA.tile([128, 512], F32, tag="psA")
                    for c in range(8):
                        nc.tensor.matmul(
                            ps,
                            lhsT=wq_b[c][:, dsl],
                            rhs=xqb_h[c // 4][:, c % 4, :],
                            start=(c == 0),
                            stop=(c == 7),
                        )
                    nc.vector.tensor_copy(qt[d][:, qsl], ps)

                if i < NKC:
                    ksl = slice(i * 512, (i + 1) * 512)
                    # KT: kt[d][:, ksl] = (xk Wk)^T
                    for d in range(2):
                        dsl = slice(d * 128, (d + 1) * 128)
                        psk = psB.tile([128, 512], F32, tag="psB")
                        for c in range(8):
                            nc.tensor.matmul(
                                psk,
                                lhsT=wk_b[c][:, dsl],
                                rhs=xkb_h[c // 4][:, c % 4, :],
                                start=(c == 0),
                                stop=(c == 7),
                            )
                        nc.vector.tensor_copy(kt[d][:, ksl], psk)
                    # Vd[kb] = xk_blk Wvd, [k, d] layout (x-block stationary)
                    for kb in range(4 * i, 4 * i + 4):
                        p4 = kb % 4
                        psv = psB.tile([128, 256], F32, tag="psB")
                        for c in range(8):
                            nc.tensor.matmul(
                                psv,
                                lhsT=xkb_h[c // 4][:, c % 4, p4 * 128 : (p4 + 1) * 128],
                                rhs=wvd_b[c],
                                start=(c == 0),
                                stop=(c == 7),
                            )
                        nc.vector.tensor_copy(vd[kb], psv)

                # ---------------- attention for qc = i ----------------
                nb = 2 * i + 2
                pts = []
                for kb in range(nb):
                    st = psA.tile([128, 512], F32, tag="psA")
                    for d in range(2):
                        nc.tensor.matmul(
                            st,
                            lhsT=kt[d][:, kb * 128 : (kb + 1) * 128],
                            rhs=qt[d][:, qsl],
                            start=(d == 0),
                            stop=(d == 1),
                        )
                    pt = ppool.tile([128, 512], BF16, tag=f"p{kb}", name=f"p{kb}")
                    nc.scalar.activation(
                        pt, st, mybir.ActivationFunctionType.Exp, scale=float(SCALE)
                    )
                    # The last two blocks straddle the causal diagonal.
                    if kb == nb - 2:
                        nc.vector.tensor_mul(pt, pt, mask_a)
                    elif kb == nb - 1:
                        nc.vector.tensor_mul(pt, pt, mask_b)
                    pts.append(pt)

                # softmax denominators: group P tiles by 4 on vector, then a
                # short ones-matmul chain: sums[1, q] += 1^T P4[k, q]
                groups = [pts[j : j + 4] for j in range(0, nb, 4)]
                grhs = []
                for gi, g in enumerate(groups):
                    if len(g) == 1:
                        grhs.append(g[0])
                    else:
                        acc = ppool.tile([128, 512], BF16, tag=f"s4_{gi}", name=f"s4_{gi}")
                        nc.vector.tensor_add(acc, g[0], g[1])
                        for t in g[2:]:
                            nc.vector.tensor_add(acc, acc, t)
                        grhs.append(acc)
                sums = psS.tile([1, 512], F32, tag="sums")
                for gi, g in enumerate(grhs):
                    nc.tensor.matmul(
                        sums,
                        lhsT=ones,
                        rhs=g,
                        start=(gi == 0),
                        stop=(gi == len(grhs) - 1),
                    )
                ssb = stage.tile([1, 512], F32, tag="ssb")
                nc.vector.tensor_copy(ssb, sums)
                nc.gpsimd.dma_start(out=ssum[i : i + 1, :], in_=ssb)

                # O'^T[d, q] += Vd[kb]^T P[kb]   (Vd blocks stationary bf16)
                ot = [
                    psOT.tile([128, 512], F32, tag=f"ot{d2}", name=f"ot{d2}")
                    for d2 in range(2)
                ]
                for kb in range(nb):
                    for d2 in range(2):
                        nc.tensor.matmul(
                            ot[d2],
                            lhsT=vd[kb][:, d2 * 128 : (d2 + 1) * 128],
                            rhs=pts[kb],
                            start=(kb == 0),
                            stop=(kb == nb - 1),
                        )
                ot_sb = []
                for d2 in range(2):
                    t = stage.tile([128, 512], BF16, tag=f"otsb{d2}", bufs=2)
                    nc.vector.tensor_copy(t, ot[d2])
                    ot_sb.append(t)

                # out[q, e] = O'[q, d] Wvu[d, e]  (O'^T blocks stationary bf16)
                for qb in range(4):
                    qbsl = slice(qb * 128, (qb + 1) * 128)
                    for eh in range(2):
                        esl = slice(eh * 512, (eh + 1) * 512)
                        po = psA.tile([128, 512], F32, tag="psA")
                        for d2 in range(2):
                            nc.tensor.matmul(
                                po,
                                lhsT=ot_sb[d2][:, qbsl],
                                rhs=wvu_b[d2][:, esl],
                                start=(d2 == 0),
                                stop=(d2 == 1),
                            )
                        ob = stage.tile([128, 512], BF16, tag="ob", bufs=3)
                        nc.scalar.activation(ob, po, mybir.ActivationFunctionType.Copy)
                        nc.gpsimd.dma_start(
                            out=o[(i * 4 + qb) * 128 : (i * 4 + qb + 1) * 128, esl],
                            in_=ob,
                        )
    nc.finalize()
    return nc


def _get_nc():
    if "nc" not in _CACHE:
        _CACHE["nc"] = _build_nc()
    return _CACHE["nc"]


def _host_masks(parity: int):
    y = np.arange(512)[None, :]
    x = np.arange(128)[:, None]
    import ml_dtypes

    mask_a = (y - x - 128 * parity >= 0).astype(ml_dtypes.bfloat16)
    mask_b = (y - x - 256 - 128 * parity >= 0).astype(ml_dtypes.bfloat16)
    return mask_a, mask_b


def _make_in_maps(inputs, Wq, Wk, Wvdown, Wvup):
    in_maps = []
    xTs = [np.ascontiguousarray(inputs[b].T) for b in range(B)]
    for core in range(NCORES):
        b, parity = core // 2, core % 2
        xb = inputs[b]  # [N, E]
        xk = np.ascontiguousarray(
            xb.reshape(N // 128, 128, E)[parity::2].reshape(KLOC, E)
        )
        xkT = np.ascontiguousarray(xk.T)  # [E, KLOC]
        mask_a, mask_b = _host_masks(parity)
        in_maps.append(
            {
                "xT": xTs[b],
                "xkT": xkT,
                "wq": Wq,
                "wk": Wk,
                "wvd": Wvdown,
                "wvu": Wvup,
                "mka": mask_a,
                "mkb": mask_b,
            }
        )
    return in_maps


def _combine(results):
    out = np.empty((B, N, E), dtype=np.float32)
    for b in range(B):
        o_sum = results[2 * b]["o"].astype(np.float32) + results[2 * b + 1][
            "o"
        ].astype(np.float32)
        s_sum = (results[2 * b]["ssum"] + results[2 * b + 1]["ssum"]).reshape(N)
        out[b] = o_sum / s_sum[:, None]
    return out


def kernel(inputs, Wq, Wk, Wvdown, Wvup):
    from concourse.bass_utils import run_bass_kernel_spmd

    inputs = np.asarray(inputs, dtype=np.float32)
    Wq = np.ascontiguousarray(np.asarray(Wq, dtype=np.float32))
    Wk = np.ascontiguousarray(np.asarray(Wk, dtype=np.float32))
    Wvdown = np.ascontiguousarray(np.asarray(Wvdown, dtype=np.float32))
    Wvup = np.ascontiguousarray(np.asarray(Wvup, dtype=np.float32))

    in_maps = _make_in_maps(inputs, Wq, Wk, Wvdown, Wvup)
    res = run_bass_kernel_spmd(_get_nc(), in_maps, core_ids=list(range(NCORES)))
    return _combine(res.results)


# revision 7
# speedup vs baseline: 1.5727x; 1.1881x over previous
"""Causal dot-product attention (low-rank V) on 8 Trainium2 NeuronCores.

Problem: inputs [B=4, N=4096, E=1024], Wq/Wk/Wvdown [E, D=256], Wvup [D, E].
    Q = x Wq; K = x Wk; S = Q K^T / sqrt(D) (causal); A = softmax(S)
    V = x Wvdown Wvup; out = A V

Sharding: core = (batch, key-parity). Each of the 4 batches is handled by a
pair of cores; core parity c owns the interleaved global key blocks {2j+c}
(128 rows each), which balances the causal work exactly. Each core produces
the *unnormalized* partial output plus softmax row-sums; the host combines
out = (O_even + O_odd) / (s_even + s_odd).

v3 design (evolved from the v1 baseline via trace analysis):
  * Low-rank reorder: out = (P^T (x Wvd)) Wvu instead of P^T (x Wvd Wvu).
    The attention contraction runs against rank-256 Vd, then one small
    [q,256]x[256,E] matmul per query chunk. Every matmul pays an exposed
    LDWEIGHTS (1 cyc/col bf16) plus free-dim stream cycles, so this cuts PE
    work ~40% vs the naive order (which reloads P blocks 8x).
  * Everything on the PE is bf16 x bf16 (the compiler rejects mixed 32/16
    bit matmuls); psum stays f32. The host pre-casts x and the weights to
    bf16, so the device does zero dtype-conversion work on x, and input DMA
    halves.
  * Projection iteration i is interleaved with attention on query chunk i:
    qc=i needs exactly key blocks 0..2i+1, which are done by iteration i.
  * All x DMA descriptors are issued in a prologue (paced by tile-pool WAR
    dependencies) so they never queue behind compute on the HWDGE engines.
  * Softmax denominators: P tiles vector-added in groups of 4, then a short
    ones-matmul chain (20 matmuls instead of 72).
  * Output partials in bf16 (halves output DMA; host combines in f32).
"""

import sys

sys.path.insert(0, "/opt/trn_rl_repo")

import numpy as np

import concourse.bacc as bacc
import concourse.mybir as mybir
import concourse.tile as tile

F32 = mybir.dt.float32
F32R = mybir.dt.float32r
BF16 = mybir.dt.bfloat16

B, N, E, D = 4, 4096, 1024, 256
NCORES = 8
KLOC = N // 2  # local keys per core
NKB = KLOC // 128  # 16 local key blocks
NQC = N // 512  # 8 query chunks of 512
NKC = KLOC // 512  # 4 local key chunks of 512
SCALE = 1.0 / np.sqrt(np.float32(D))  # 1/16

_CACHE = {}


def _build_nc(reps=1):
    nc = bacc.Bacc("TRN2", target_bir_lowering=False)

    xTb = nc.dram_tensor("xTb", [E, N], BF16, kind="ExternalInput")
    xkTb = nc.dram_tensor("xkTb", [E, KLOC], BF16, kind="ExternalInput")
    wq = nc.dram_tensor("wq", [E, D], BF16, kind="ExternalInput")
    wk = nc.dram_tensor("wk", [E, D], BF16, kind="ExternalInput")
    wvd = nc.dram_tensor("wvd", [E, D], BF16, kind="ExternalInput")
    wvu = nc.dram_tensor("wvu", [D, E], BF16, kind="ExternalInput")
    mka = nc.dram_tensor("mka", [128, 512], BF16, kind="ExternalInput")
    mkb = nc.dram_tensor("mkb", [128, 512], BF16, kind="ExternalInput")

    o = nc.dram_tensor("o", [N, E], BF16, kind="ExternalOutput")
    ssum = nc.dram_tensor("ssum", [NQC, 512], F32, kind="ExternalOutput")

    with tile.TileContext(nc) as tc:
      for _rep in range(reps):
        with (
            tc.tile_pool(name=f"res{_rep}", bufs=1) as res,
            tc.tile_pool(name=f"consts{_rep}", bufs=1) as consts,
            tc.tile_pool(name=f"wpool{_rep}", bufs=1) as wp,
            tc.tile_pool(name=f"xstream{_rep}", bufs=2) as xs,
            tc.tile_pool(name=f"ppool{_rep}", bufs=1) as ppool,
            tc.tile_pool(name=f"stage{_rep}", bufs=3) as stage,
            tc.tile_pool(name=f"psA{_rep}", bufs=3, space="PSUM") as psA,
            tc.tile_pool(name=f"psB{_rep}", bufs=2, space="PSUM") as psB,
            tc.tile_pool(name=f"psOT{_rep}", bufs=1, space="PSUM") as psOT,
            tc.tile_pool(name=f"psS{_rep}", bufs=1, space="PSUM") as psS,
        ):
            # ---- residents ----
            qt = [res.tile([128, N], BF16, tag=f"qt{d}", name=f"qt{d}") for d in range(2)]
            kt = [res.tile([128, KLOC], BF16, tag=f"kt{d}", name=f"kt{d}") for d in range(2)]
            vd = [res.tile([128, D], BF16, tag=f"vd{kb}", name=f"vd{kb}") for kb in range(NKB)]

            ones_f = consts.tile([128, 1], F32, tag="ones_f")
            nc.vector.memset(ones_f, 1.0)
            ones = consts.tile([128, 1], BF16, tag="ones")
            nc.vector.tensor_copy(ones, ones_f)
            mask_a = consts.tile([128, 512], BF16, tag="mka")
            mask_b = consts.tile([128, 512], BF16, tag="mkb")

            # ---- weight DMAs (gpsimd queue), already bf16 on host ----
            wq_b = [wp.tile([128, D], BF16, tag=f"wqb{c}", name=f"wqb{c}") for c in range(8)]
            wk_b = [wp.tile([128, D], BF16, tag=f"wkb{c}", name=f"wkb{c}") for c in range(8)]
            wvd_b = [wp.tile([128, D], BF16, tag=f"wvdb{c}", name=f"wvdb{c}") for c in range(8)]
            wvu_b = [wp.tile([128, E], BF16, tag=f"wvub{d}", name=f"wvub{d}") for d in range(2)]
            for c in range(8):
                sl = slice(c * 128, (c + 1) * 128)
                nc.gpsimd.dma_start(out=wq_b[c], in_=wq[sl, :])
                nc.gpsimd.dma_start(out=wk_b[c], in_=wk[sl, :])
            for c in range(8):
                sl = slice(c * 128, (c + 1) * 128)
                nc.gpsimd.dma_start(out=wvd_b[c], in_=wvd[sl, :])
            for d in range(2):
                nc.gpsimd.dma_start(out=wvu_b[d], in_=wvu[d * 128 : (d + 1) * 128, :])
            nc.gpsimd.dma_start(out=mask_a, in_=mka[:, :])
            nc.gpsimd.dma_start(out=mask_b, in_=mkb[:, :])

            # ---- x DMA prologue: all descriptors issued up front; the
            # xstream pool (bufs=2) paces transfers via WAR deps. ----
            xq_t = [[None] * NQC, [None] * NQC]
            xk_t = [[None] * NKC, [None] * NKC]
            for i in range(NQC):
                for h, eng in ((0, nc.sync), (1, nc.scalar)):
                    xqh = xs.tile([128, 4, 512], BF16, tag=f"xq{h}", bufs=2, name=f"xq{h}_{i}")
                    eng.dma_start(
                        out=xqh,
                        in_=xTb[
                            h * 512 : (h + 1) * 512, i * 512 : (i + 1) * 512
                        ].rearrange("(c p) q -> p c q", p=128),
                    )
                    xq_t[h][i] = xqh
                if i < NKC:
                    for h, eng in ((0, nc.scalar), (1, nc.sync)):
                        xkh = xs.tile([128, 4, 512], BF16, tag=f"xk{h}", bufs=2, name=f"xk{h}_{i}")
                        eng.dma_start(
                            out=xkh,
                            in_=xkTb[
                                h * 512 : (h + 1) * 512, i * 512 : (i + 1) * 512
                            ].rearrange("(c p) q -> p c q", p=128),
                        )
                        xk_t[h][i] = xkh

            # ---- merged loop: projections(i) + attention(qc=i) ----
            for i in range(NQC):
                qsl = slice(i * 512, (i + 1) * 512)
                xq_h = [xq_t[0][i], xq_t[1][i]]

                # QT: qt[d][:, qsl] = (x Wq)^T
                for d in range(2):
                    dsl = slice(d * 128, (d + 1) * 128)
                    ps = psA.tile([128, 512], F32, tag="psA")
                    for c in range(8):
                        nc.tensor.matmul(
                            ps,
                            lhsT=wq_b[c][:, dsl],
                            rhs=xq_h[c // 4][:, c % 4, :],
                            start=(c == 0),
                            stop=(c == 7),
                        )
                    nc.vector.tensor_copy(qt[d][:, qsl], ps)

                if i < NKC:
                    xk_h = [xk_t[0][i], xk_t[1][i]]
                    ksl = slice(i * 512, (i + 1) * 512)
                    # KT: kt[d][:, ksl] = (xk Wk)^T
                    for d in range(2):
                        dsl = slice(d * 128, (d + 1) * 128)
                        psk = psB.tile([128, 512], F32, tag="psB")
                        for c in range(8):
                            nc.tensor.matmul(
                                psk,
                                lhsT=wk_b[c][:, dsl],
                                rhs=xk_h[c // 4][:, c % 4, :],
                                start=(c == 0),
                                stop=(c == 7),
                            )
                        nc.vector.tensor_copy(kt[d][:, ksl], psk)
                    # Vd[kb] = xk_blk Wvd, [k, d] layout (x-block stationary)
                    for kb in range(4 * i, 4 * i + 4):
                        p4 = kb % 4
                        psv = psB.tile([128, 256], F32, tag="psB")
                        for c in range(8):
                            nc.tensor.matmul(
                                psv,
                                lhsT=xk_h[c // 4][:, c % 4, p4 * 128 : (p4 + 1) * 128],
                                rhs=wvd_b[c],
                                start=(c == 0),
                                stop=(c == 7),
                            )
                        nc.vector.tensor_copy(vd[kb], psv)

                # ---------------- attention for qc = i ----------------
                nb = 2 * i + 2
                pts = []
                for kb in range(nb):
                    st = psA.tile([128, 512], F32, tag="psA")
                    for d in range(2):
                        nc.tensor.matmul(
                            st,
                            lhsT=kt[d][:, kb * 128 : (kb + 1) * 128],
                            rhs=qt[d][:, qsl],
                            start=(d == 0),
                            stop=(d == 1),
                        )
                    pt = ppool.tile([128, 512], BF16, tag=f"p{kb}", name=f"p{kb}")
                    nc.scalar.activation(
                        pt, st, mybir.ActivationFunctionType.Exp, scale=float(SCALE)
                    )
                    # The last two blocks straddle the causal diagonal.
                    if kb == nb - 2:
                        nc.vector.tensor_mul(pt, pt, mask_a)
                    elif kb == nb - 1:
                        nc.vector.tensor_mul(pt, pt, mask_b)
                    pts.append(pt)

                # softmax denominators: group P tiles by 4 on vector, then a
                # short ones-matmul chain: sums[1, q] += 1^T P4[k, q]
                groups = [pts[j : j + 4] for j in range(0, nb, 4)]
                grhs = []
                for gi, g in enumerate(groups):
                    if len(g) == 1:
                        grhs.append(g[0])
                    else:
                        acc = ppool.tile([128, 512], BF16, tag=f"s4_{gi}", name=f"s4_{gi}")
                        nc.vector.tensor_add(acc, g[0], g[1])
                        for t in g[2:]:
                            nc.vector.tensor_add(acc, acc, t)
                        grhs.append(acc)
                sums = psS.tile([1, 512], F32, tag="sums")
                for gi, g in enumerate(grhs):
                    nc.tensor.matmul(
                        sums,
                        lhsT=ones,
                        rhs=g,
                        start=(gi == 0),
                        stop=(gi == len(grhs) - 1),
                    )
                ssb = stage.tile([1, 512], F32, tag="ssb")
                nc.vector.tensor_copy(ssb, sums)
                nc.gpsimd.dma_start(out=ssum[i : i + 1, :], in_=ssb)

                # O'^T[d, q] += Vd[kb]^T P[kb]   (Vd blocks stationary)
                ot = [
                    psOT.tile([128, 512], F32, tag=f"ot{d2}", name=f"ot{d2}")
                    for d2 in range(2)
                ]
                for kb in range(nb):
                    for d2 in range(2):
                        nc.tensor.matmul(
                            ot[d2],
                            lhsT=vd[kb][:, d2 * 128 : (d2 + 1) * 128],
                            rhs=pts[kb],
                            start=(kb == 0),
                            stop=(kb == nb - 1),
                        )
                ot_sb = []
                for d2 in range(2):
                    t = stage.tile([128, 512], BF16, tag=f"otsb{d2}", bufs=2)
                    nc.vector.tensor_copy(t, ot[d2])
                    ot_sb.append(t)

                # out[q, e] = O'[q, d] Wvu[d, e]  (O'^T blocks stationary)
                for qb in range(4):
                    qbsl = slice(qb * 128, (qb + 1) * 128)
                    for eh in range(2):
                        esl = slice(eh * 512, (eh + 1) * 512)
                        po = psA.tile([128, 512], F32, tag="psA")
                        for d2 in range(2):
                            nc.tensor.matmul(
                                po,
                                lhsT=ot_sb[d2][:, qbsl],
                                rhs=wvu_b[d2][:, esl],
                                start=(d2 == 0),
                                stop=(d2 == 1),
                            )
                        ob = stage.tile([128, 512], BF16, tag="ob", bufs=4)
                        nc.scalar.activation(ob, po, mybir.ActivationFunctionType.Copy)
                        nc.gpsimd.dma_start(
                            out=o[(i * 4 + qb) * 128 : (i * 4 + qb + 1) * 128, esl],
                            in_=ob,
                        )
    nc.finalize()
    return nc


def _get_nc():
    if "nc" not in _CACHE:
        _CACHE["nc"] = _build_nc()
    return _CACHE["nc"]


def _host_masks(parity: int):
    import ml_dtypes

    y = np.arange(512)[None, :]
    x = np.arange(128)[:, None]
    mask_a = (y - x - 128 * parity >= 0).astype(ml_dtypes.bfloat16)
    mask_b = (y - x - 256 - 128 * parity >= 0).astype(ml_dtypes.bfloat16)
    return mask_a, mask_b


def _make_in_maps(inputs, Wq, Wk, Wvdown, Wvup):
    import ml_dtypes

    bf16 = ml_dtypes.bfloat16
    in_maps = []
    wq_b = np.ascontiguousarray(np.asarray(Wq, np.float32).astype(bf16))
    wk_b = np.ascontiguousarray(np.asarray(Wk, np.float32).astype(bf16))
    wvd_b = np.ascontiguousarray(np.asarray(Wvdown, np.float32).astype(bf16))
    wvu_b = np.ascontiguousarray(np.asarray(Wvup, np.float32).astype(bf16))
    xb16 = [np.asarray(inputs[b], np.float32).astype(bf16) for b in range(B)]
    xTs = [np.ascontiguousarray(xb16[b].T) for b in range(B)]
    for core in range(NCORES):
        b, parity = core // 2, core % 2
        xk = np.ascontiguousarray(
            xb16[b].reshape(N // 128, 128, E)[parity::2].reshape(KLOC, E)
        )
        xkT = np.ascontiguousarray(xk.T)  # [E, KLOC] bf16
        mask_a, mask_b = _host_masks(parity)
        in_maps.append(
            {
                "xTb": xTs[b],
                "xkTb": xkT,
                "wq": wq_b,
                "wk": wk_b,
                "wvd": wvd_b,
                "wvu": wvu_b,
                "mka": mask_a,
                "mkb": mask_b,
            }
        )
    return in_maps


def _combine(results):
    out = np.empty((B, N, E), dtype=np.float32)
    for b in range(B):
        o_sum = results[2 * b]["o"].astype(np.float32) + results[2 * b + 1][
            "o"
        ].astype(np.float32)
        s_sum = (results[2 * b]["ssum"] + results[2 * b + 1]["ssum"]).reshape(N)
        out[b] = o_sum / s_sum[:, None]
    return out


def kernel(inputs, Wq, Wk, Wvdown, Wvup):
    from concourse.bass_utils import run_bass_kernel_spmd

    inputs = np.asarray(inputs, dtype=np.float32)
    in_maps = _make_in_maps(inputs, Wq, Wk, Wvdown, Wvup)
    res = run_bass_kernel_spmd(_get_nc(), in_maps, core_ids=list(range(NCORES)))
    return _combine(res.results)


# revision 8
# speedup vs baseline: 1.7655x; 1.1226x over previous
"""Causal dot-product attention (low-rank V) on 8 Trainium2 NeuronCores.

Problem: inputs [B=4, N=4096, E=1024], Wq/Wk/Wvdown [E, D=256], Wvup [D, E].
    Q = x Wq; K = x Wk; S = Q K^T / sqrt(D) (causal); A = softmax(S)
    V = x Wvdown Wvup; out = A V

Sharding: core = (batch, key-parity). Each of the 4 batches is handled by a
pair of cores; core parity c owns the interleaved global key blocks {2j+c}
(128 rows each), which balances the causal work exactly. Each core produces
the *unnormalized* partial output plus softmax row-sums; the host combines
out = (O_even + O_odd) / (s_even + s_odd).

v3 design (evolved from the v1 baseline via trace analysis):
  * Low-rank reorder: out = (P^T (x Wvd)) Wvu instead of P^T (x Wvd Wvu).
    The attention contraction runs against rank-256 Vd, then one small
    [q,256]x[256,E] matmul per query chunk. Every matmul pays an exposed
    LDWEIGHTS (1 cyc/col bf16) plus free-dim stream cycles, so this cuts PE
    work ~40% vs the naive order (which reloads P blocks 8x).
  * Everything on the PE is bf16 x bf16 (the compiler rejects mixed 32/16
    bit matmuls); psum stays f32. The host pre-casts x and the weights to
    bf16, so the device does zero dtype-conversion work on x, and input DMA
    halves.
  * Projection iteration i is interleaved with attention on query chunk i:
    qc=i needs exactly key blocks 0..2i+1, which are done by iteration i.
  * All x DMA descriptors are issued in a prologue (paced by tile-pool WAR
    dependencies) so they never queue behind compute on the HWDGE engines.
  * Softmax denominators: P tiles vector-added in groups of 4, then a short
    ones-matmul chain (20 matmuls instead of 72).
  * Output partials in bf16 (halves output DMA; host combines in f32).
"""

import sys

sys.path.insert(0, "/opt/trn_rl_repo")

import numpy as np

import concourse.bacc as bacc
import concourse.mybir as mybir
import concourse.tile as tile

F32 = mybir.dt.float32
F32R = mybir.dt.float32r
BF16 = mybir.dt.bfloat16

B, N, E, D = 4, 4096, 1024, 256
NCORES = 8
KLOC = N // 2  # local keys per core
NKB = KLOC // 128  # 16 local key blocks
NQC = N // 512  # 8 query chunks of 512
NKC = KLOC // 512  # 4 local key chunks of 512
SCALE = 1.0 / np.sqrt(np.float32(D))  # 1/16

_CACHE = {}


def _build_nc(reps=1):
    nc = bacc.Bacc("TRN2", target_bir_lowering=False)

    xTb = nc.dram_tensor("xTb", [E, N], BF16, kind="ExternalInput")
    xkTb = nc.dram_tensor("xkTb", [E, KLOC], BF16, kind="ExternalInput")
    wq = nc.dram_tensor("wq", [E, D], BF16, kind="ExternalInput")
    wk = nc.dram_tensor("wk", [E, D], BF16, kind="ExternalInput")
    wvd = nc.dram_tensor("wvd", [E, D], BF16, kind="ExternalInput")
    wvu = nc.dram_tensor("wvu", [D, E], BF16, kind="ExternalInput")
    mka = nc.dram_tensor("mka", [128, 512], BF16, kind="ExternalInput")
    mkb = nc.dram_tensor("mkb", [128, 512], BF16, kind="ExternalInput")

    o = nc.dram_tensor("o", [N, E], BF16, kind="ExternalOutput")
    ssum = nc.dram_tensor("ssum", [NQC, 512], F32, kind="ExternalOutput")

    with tile.TileContext(nc) as tc:
      for _rep in range(reps):
        with (
            tc.tile_pool(name=f"res{_rep}", bufs=1) as res,
            tc.tile_pool(name=f"consts{_rep}", bufs=1) as consts,
            tc.tile_pool(name=f"wpool{_rep}", bufs=1) as wp,
            tc.tile_pool(name=f"xstream{_rep}", bufs=2) as xs,
            tc.tile_pool(name=f"ppool{_rep}", bufs=1) as ppool,
            tc.tile_pool(name=f"stage{_rep}", bufs=3) as stage,
            tc.tile_pool(name=f"psA{_rep}", bufs=3, space="PSUM") as psA,
            tc.tile_pool(name=f"psB{_rep}", bufs=2, space="PSUM") as psB,
            tc.tile_pool(name=f"psOT{_rep}", bufs=1, space="PSUM") as psOT,
            tc.tile_pool(name=f"psS{_rep}", bufs=1, space="PSUM") as psS,
        ):
            # ---- residents ----
            qt = [res.tile([128, N], BF16, tag=f"qt{d}", name=f"qt{d}") for d in range(2)]
            kt = [res.tile([128, KLOC], BF16, tag=f"kt{d}", name=f"kt{d}") for d in range(2)]
            vd = [res.tile([128, D], BF16, tag=f"vd{kb}", name=f"vd{kb}") for kb in range(NKB)]

            ones_f = consts.tile([128, 1], F32, tag="ones_f")
            nc.vector.memset(ones_f, 1.0)
            ones = consts.tile([128, 1], BF16, tag="ones")
            nc.vector.tensor_copy(ones, ones_f)
            mask_a = consts.tile([128, 512], BF16, tag="mka")
            mask_b = consts.tile([128, 512], BF16, tag="mkb")

            # ---- weight DMAs (gpsimd queue), already bf16 on host ----
            wq_b = [wp.tile([128, D], BF16, tag=f"wqb{c}", name=f"wqb{c}") for c in range(8)]
            wk_b = [wp.tile([128, D], BF16, tag=f"wkb{c}", name=f"wkb{c}") for c in range(8)]
            wvd_b = [wp.tile([128, D], BF16, tag=f"wvdb{c}", name=f"wvdb{c}") for c in range(8)]
            wvu_b = [wp.tile([128, E], BF16, tag=f"wvub{d}", name=f"wvub{d}") for d in range(2)]
            for c in range(8):
                sl = slice(c * 128, (c + 1) * 128)
                nc.gpsimd.dma_start(out=wq_b[c], in_=wq[sl, :])
                nc.gpsimd.dma_start(out=wk_b[c], in_=wk[sl, :])
            for c in range(8):
                sl = slice(c * 128, (c + 1) * 128)
                nc.gpsimd.dma_start(out=wvd_b[c], in_=wvd[sl, :])
            for d in range(2):
                nc.gpsimd.dma_start(out=wvu_b[d], in_=wvu[d * 128 : (d + 1) * 128, :])
            nc.gpsimd.dma_start(out=mask_a, in_=mka[:, :])
            nc.gpsimd.dma_start(out=mask_b, in_=mkb[:, :])

            # ---- x DMA prologue: all descriptors issued up front; the
            # xstream pool (bufs=2) paces transfers via WAR deps. ----
            xq_t = [[None] * NQC, [None] * NQC]
            xk_t = [[None] * NKC, [None] * NKC]
            for i in range(NQC):
                for h, eng in ((0, nc.sync), (1, nc.scalar)):
                    xqh = xs.tile([128, 4, 512], BF16, tag=f"xq{h}", bufs=2, name=f"xq{h}_{i}")
                    eng.dma_start(
                        out=xqh,
                        in_=xTb[
                            h * 512 : (h + 1) * 512, i * 512 : (i + 1) * 512
                        ].rearrange("(c p) q -> p c q", p=128),
                    )
                    xq_t[h][i] = xqh
                if i < NKC:
                    for h, eng in ((0, nc.scalar), (1, nc.sync)):
                        xkh = xs.tile([128, 4, 512], BF16, tag=f"xk{h}", bufs=2, name=f"xk{h}_{i}")
                        eng.dma_start(
                            out=xkh,
                            in_=xkTb[
                                h * 512 : (h + 1) * 512, i * 512 : (i + 1) * 512
                            ].rearrange("(c p) q -> p c q", p=128),
                        )
                        xk_t[h][i] = xkh

            # ---- merged loop: projections(i) + attention(qc=i) ----
            for i in range(NQC):
                qsl = slice(i * 512, (i + 1) * 512)
                xq_h = [xq_t[0][i], xq_t[1][i]]

                # QT: qt[d][:, qsl] = (x Wq)^T
                for d in range(2):
                    dsl = slice(d * 128, (d + 1) * 128)
                    ps = psA.tile([128, 512], F32, tag="psA")
                    for c in range(8):
                        nc.tensor.matmul(
                            ps,
                            lhsT=wq_b[c][:, dsl],
                            rhs=xq_h[c // 4][:, c % 4, :],
                            start=(c == 0),
                            stop=(c == 7),
                        )
                    nc.vector.tensor_copy(qt[d][:, qsl], ps)

                if i < NKC:
                    xk_h = [xk_t[0][i], xk_t[1][i]]
                    ksl = slice(i * 512, (i + 1) * 512)
                    # KT: kt[d][:, ksl] = (xk Wk)^T
                    for d in range(2):
                        dsl = slice(d * 128, (d + 1) * 128)
                        psk = psB.tile([128, 512], F32, tag="psB")
                        for c in range(8):
                            nc.tensor.matmul(
                                psk,
                                lhsT=wk_b[c][:, dsl],
                                rhs=xk_h[c // 4][:, c % 4, :],
                                start=(c == 0),
                                stop=(c == 7),
                            )
                        nc.vector.tensor_copy(kt[d][:, ksl], psk)
                    # Vd[kb] = xk_blk Wvd, [k, d] layout (x-block stationary)
                    for kb in range(4 * i, 4 * i + 4):
                        p4 = kb % 4
                        psv = psB.tile([128, 256], F32, tag="psB")
                        for c in range(8):
                            nc.tensor.matmul(
                                psv,
                                lhsT=xk_h[c // 4][:, c % 4, p4 * 128 : (p4 + 1) * 128],
                                rhs=wvd_b[c],
                                start=(c == 0),
                                stop=(c == 7),
                            )
                        nc.vector.tensor_copy(vd[kb], psv)

                # ---------------- attention for qc = i ----------------
                nb = 2 * i + 2
                pts = []
                for kb in range(nb):
                    st = psA.tile([128, 512], F32, tag="psA")
                    for d in range(2):
                        nc.tensor.matmul(
                            st,
                            lhsT=kt[d][:, kb * 128 : (kb + 1) * 128],
                            rhs=qt[d][:, qsl],
                            start=(d == 0),
                            stop=(d == 1),
                        )
                    pt = ppool.tile([128, 512], BF16, tag=f"p{kb}", name=f"p{kb}")
                    nc.scalar.activation(
                        pt, st, mybir.ActivationFunctionType.Exp, scale=float(SCALE)
                    )
                    # The last two blocks straddle the causal diagonal.
                    if kb == nb - 2:
                        nc.vector.tensor_mul(pt, pt, mask_a)
                    elif kb == nb - 1:
                        nc.vector.tensor_mul(pt, pt, mask_b)
                    pts.append(pt)

                # softmax denominator prep: group P tiles by 4 on vector;
                # these adds overlap the O'T matmul burst below.
                groups = [pts[j : j + 4] for j in range(0, nb, 4)]
                grhs = []
                for gi, g in enumerate(groups):
                    if len(g) == 1:
                        grhs.append(g[0])
                    else:
                        acc = ppool.tile([128, 512], BF16, tag=f"s4_{gi}", name=f"s4_{gi}")
                        nc.vector.tensor_add(acc, g[0], g[1])
                        for t in g[2:]:
                            nc.vector.tensor_add(acc, acc, t)
                        grhs.append(acc)

                # O'^T[d, q] += Vd[kb]^T P[kb]   (Vd blocks stationary)
                ot = [
                    psOT.tile([128, 512], F32, tag=f"ot{d2}", name=f"ot{d2}")
                    for d2 in range(2)
                ]
                for kb in range(nb):
                    for d2 in range(2):
                        nc.tensor.matmul(
                            ot[d2],
                            lhsT=vd[kb][:, d2 * 128 : (d2 + 1) * 128],
                            rhs=pts[kb],
                            start=(kb == 0),
                            stop=(kb == nb - 1),
                        )
                ot_sb = []
                for d2 in range(2):
                    t = stage.tile([128, 512], BF16, tag=f"otsb{d2}", bufs=2)
                    nc.vector.tensor_copy(t, ot[d2])
                    ot_sb.append(t)

                # sums[1, q] += 1^T P4[k, q]  (inputs ready: adds overlapped)
                sums = psS.tile([1, 512], F32, tag="sums")
                for gi, g in enumerate(grhs):
                    nc.tensor.matmul(
                        sums,
                        lhsT=ones,
                        rhs=g,
                        start=(gi == 0),
                        stop=(gi == len(grhs) - 1),
                    )
                ssb = stage.tile([1, 512], F32, tag="ssb")
                nc.vector.tensor_copy(ssb, sums)
                nc.gpsimd.dma_start(out=ssum[i : i + 1, :], in_=ssb)

                # out[q, e] = O'[q, d] Wvu[d, e]  (O'^T blocks stationary)
                for qb in range(4):
                    qbsl = slice(qb * 128, (qb + 1) * 128)
                    for eh in range(2):
                        esl = slice(eh * 512, (eh + 1) * 512)
                        po = psA.tile([128, 512], F32, tag="psA")
                        for d2 in range(2):
                            nc.tensor.matmul(
                                po,
                                lhsT=ot_sb[d2][:, qbsl],
                                rhs=wvu_b[d2][:, esl],
                                start=(d2 == 0),
                                stop=(d2 == 1),
                            )
                        ob = stage.tile([128, 512], BF16, tag=f"ob{eh}", bufs=4, name=f"ob{eh}")
                        if eh == 0:
                            nc.vector.tensor_copy(ob, po)
                        else:
                            nc.scalar.activation(
                                ob, po, mybir.ActivationFunctionType.Copy
                            )
                        nc.sync.dma_start(
                            out=o[(i * 4 + qb) * 128 : (i * 4 + qb + 1) * 128, esl],
                            in_=ob,
                        )
    nc.finalize()
    return nc


def _get_nc():
    if "nc" not in _CACHE:
        _CACHE["nc"] = _build_nc()
    return _CACHE["nc"]


def _host_masks(parity: int):
    import ml_dtypes

    y = np.arange(512)[None, :]
    x = np.arange(128)[:, None]
    mask_a = (y - x - 128 * parity >= 0).astype(ml_dtypes.bfloat16)
    mask_b = (y - x - 256 - 128 * parity >= 0).astype(ml_dtypes.bfloat16)
    return mask_a, mask_b


def _make_in_maps(inputs, Wq, Wk, Wvdown, Wvup):
    import ml_dtypes

    bf16 = ml_dtypes.bfloat16
    in_maps = []
    wq_b = np.ascontiguousarray(np.asarray(Wq, np.float32).astype(bf16))
    wk_b = np.ascontiguousarray(np.asarray(Wk, np.float32).astype(bf16))
    wvd_b = np.ascontiguousarray(np.asarray(Wvdown, np.float32).astype(bf16))
    wvu_b = np.ascontiguousarray(np.asarray(Wvup, np.float32).astype(bf16))
    xb16 = [np.asarray(inputs[b], np.float32).astype(bf16) for b in range(B)]
    xTs = [np.ascontiguousarray(xb16[b].T) for b in range(B)]
    for core in range(NCORES):
        b, parity = core // 2, core % 2
        xk = np.ascontiguousarray(
            xb16[b].reshape(N // 128, 128, E)[parity::2].reshape(KLOC, E)
        )
        xkT = np.ascontiguousarray(xk.T)  # [E, KLOC] bf16
        mask_a, mask_b = _host_masks(parity)
        in_maps.append(
            {
                "xTb": xTs[b],
                "xkTb": xkT,
                "wq": wq_b,
                "wk": wk_b,
                "wvd": wvd_b,
                "wvu": wvu_b,
                "mka": mask_a,
                "mkb": mask_b,
            }
        )
    return in_maps


def _combine(results):
    out = np.empty((B, N, E), dtype=np.float32)
    for b in range(B):
        o_sum = results[2 * b]["o"].astype(np.float32) + results[2 * b + 1][
            "o"
        ].astype(np.float32)
        s_sum = (results[2 * b]["ssum"] + results[2 * b + 1]["ssum"]).reshape(N)
        out[b] = o_sum / s_sum[:, None]
    return out


def kernel(inputs, Wq, Wk, Wvdown, Wvup):
    from concourse.bass_utils import run_bass_kernel_spmd

    inputs = np.asarray(inputs, dtype=np.float32)
    in_maps = _make_in_maps(inputs, Wq, Wk, Wvdown, Wvup)
    res = run_bass_kernel_spmd(_get_nc(), in_maps, core_ids=list(range(NCORES)))
    return _combine(res.results)


# revision 9
# speedup vs baseline: 1.7930x; 1.0155x over previous
"""Causal dot-product attention (low-rank V) on 8 Trainium2 NeuronCores.

Problem: inputs [B=4, N=4096, E=1024], Wq/Wk/Wvdown [E, D=256], Wvup [D, E].
    Q = x Wq; K = x Wk; S = Q K^T / sqrt(D) (causal); A = softmax(S)
    V = x Wvdown Wvup; out = A V

Sharding: core = (batch, key-parity). Each of the 4 batches is handled by a
pair of cores; core parity c owns the interleaved global key blocks {2j+c}
(128 rows each), which balances the causal work exactly. Each core produces
the *unnormalized* partial output plus softmax row-sums; the host combines
out = (O_even + O_odd) / (s_even + s_odd).

v3 design (evolved from the v1 baseline via trace analysis):
  * Low-rank reorder: out = (P^T (x Wvd)) Wvu instead of P^T (x Wvd Wvu).
    The attention contraction runs against rank-256 Vd, then one small
    [q,256]x[256,E] matmul per query chunk. Every matmul pays an exposed
    LDWEIGHTS (1 cyc/col bf16) plus free-dim stream cycles, so this cuts PE
    work ~40% vs the naive order (which reloads P blocks 8x).
  * Everything on the PE is bf16 x bf16 (the compiler rejects mixed 32/16
    bit matmuls); psum stays f32. The host pre-casts x and the weights to
    bf16, so the device does zero dtype-conversion work on x, and input DMA
    halves.
  * Projection iteration i is interleaved with attention on query chunk i:
    qc=i needs exactly key blocks 0..2i+1, which are done by iteration i.
  * All x DMA descriptors are issued in a prologue (paced by tile-pool WAR
    dependencies) so they never queue behind compute on the HWDGE engines.
  * Softmax denominators: P tiles vector-added in groups of 4, then a short
    ones-matmul chain (20 matmuls instead of 72).
  * Output partials in bf16 (halves output DMA; host combines in f32).
"""

import sys

sys.path.insert(0, "/opt/trn_rl_repo")

import numpy as np

import concourse.bacc as bacc
import concourse.mybir as mybir
import concourse.tile as tile

F32 = mybir.dt.float32
F32R = mybir.dt.float32r
BF16 = mybir.dt.bfloat16

B, N, E, D = 4, 4096, 1024, 256
NCORES = 8
KLOC = N // 2  # local keys per core
NKB = KLOC // 128  # 16 local key blocks
NQC = N // 512  # 8 query chunks of 512
NKC = KLOC // 512  # 4 local key chunks of 512
SCALE = 1.0 / np.sqrt(np.float32(D))  # 1/16

_CACHE = {}


def _build_nc(reps=1):
    nc = bacc.Bacc("TRN2", target_bir_lowering=False)

    xTb = nc.dram_tensor("xTb", [E, N], BF16, kind="ExternalInput")
    xkTb = nc.dram_tensor("xkTb", [E, KLOC], BF16, kind="ExternalInput")
    wq = nc.dram_tensor("wq", [E, D], BF16, kind="ExternalInput")
    wk = nc.dram_tensor("wk", [E, D], BF16, kind="ExternalInput")
    wvd = nc.dram_tensor("wvd", [E, D], BF16, kind="ExternalInput")
    wvu = nc.dram_tensor("wvu", [D, E], BF16, kind="ExternalInput")
    mka = nc.dram_tensor("mka", [128, 512], BF16, kind="ExternalInput")
    mkb = nc.dram_tensor("mkb", [128, 512], BF16, kind="ExternalInput")

    o = nc.dram_tensor("o", [N, E], BF16, kind="ExternalOutput")
    ssum = nc.dram_tensor("ssum", [NQC, 512], F32, kind="ExternalOutput")

    with tile.TileContext(nc) as tc:
      for _rep in range(reps):
        with (
            tc.tile_pool(name=f"res{_rep}", bufs=1) as res,
            tc.tile_pool(name=f"consts{_rep}", bufs=1) as consts,
            tc.tile_pool(name=f"wpool{_rep}", bufs=1) as wp,
            tc.tile_pool(name=f"xstream{_rep}", bufs=2) as xs,
            tc.tile_pool(name=f"ppool{_rep}", bufs=1) as ppool,
            tc.tile_pool(name=f"stage{_rep}", bufs=3) as stage,
            tc.tile_pool(name=f"psA{_rep}", bufs=3, space="PSUM") as psA,
            tc.tile_pool(name=f"psB{_rep}", bufs=2, space="PSUM") as psB,
            tc.tile_pool(name=f"psOT{_rep}", bufs=1, space="PSUM") as psOT,
            tc.tile_pool(name=f"psS{_rep}", bufs=1, space="PSUM") as psS,
        ):
            # ---- residents ----
            qt = [res.tile([128, N], BF16, tag=f"qt{d}", name=f"qt{d}") for d in range(2)]
            kt = [res.tile([128, KLOC], BF16, tag=f"kt{d}", name=f"kt{d}") for d in range(2)]
            vd = [res.tile([128, D], BF16, tag=f"vd{kb}", name=f"vd{kb}") for kb in range(NKB)]

            ones_f = consts.tile([128, 1], F32, tag="ones_f")
            nc.vector.memset(ones_f, 1.0)
            ones = consts.tile([128, 1], BF16, tag="ones")
            nc.vector.tensor_copy(ones, ones_f)
            mask_a = consts.tile([128, 512], BF16, tag="mka")
            mask_b = consts.tile([128, 512], BF16, tag="mkb")

            # ---- weight DMAs: one descriptor per matrix (gpsimd queue) ----
            wqB = wp.tile([128, 8, D], BF16, tag="wqB")
            wkB = wp.tile([128, 8, D], BF16, tag="wkB")
            wvdB = wp.tile([128, 8, D], BF16, tag="wvdB")
            wvuB = wp.tile([128, 2, E], BF16, tag="wvuB")
            nc.gpsimd.dma_start(out=wqB, in_=wq.rearrange("(c p) d -> p c d", p=128))
            nc.gpsimd.dma_start(out=wkB, in_=wk.rearrange("(c p) d -> p c d", p=128))
            nc.gpsimd.dma_start(out=wvdB, in_=wvd.rearrange("(c p) d -> p c d", p=128))
            nc.gpsimd.dma_start(out=wvuB, in_=wvu.rearrange("(c p) d -> p c d", p=128))
            nc.gpsimd.dma_start(out=mask_a, in_=mka[:, :])
            nc.gpsimd.dma_start(out=mask_b, in_=mkb[:, :])
            wq_b = [wqB[:, c, :] for c in range(8)]
            wk_b = [wkB[:, c, :] for c in range(8)]
            wvd_b = [wvdB[:, c, :] for c in range(8)]
            wvu_b = [wvuB[:, d, :] for d in range(2)]

            # ---- x DMA prologue: all descriptors issued up front; the
            # xstream pool (bufs=2) paces transfers via WAR deps. ----
            xq_t = [[None] * NQC, [None] * NQC]
            xk_t = [[None] * NKC, [None] * NKC]
            for i in range(NQC):
                for h, eng in ((0, nc.sync), (1, nc.scalar)):
                    xqh = xs.tile([128, 4, 512], BF16, tag=f"xq{h}", bufs=3, name=f"xq{h}_{i}")
                    eng.dma_start(
                        out=xqh,
                        in_=xTb[
                            h * 512 : (h + 1) * 512, i * 512 : (i + 1) * 512
                        ].rearrange("(c p) q -> p c q", p=128),
                    )
                    xq_t[h][i] = xqh
                if i < NKC:
                    for h, eng in ((0, nc.scalar), (1, nc.sync)):
                        xkh = xs.tile([128, 4, 512], BF16, tag=f"xk{h}", bufs=3, name=f"xk{h}_{i}")
                        eng.dma_start(
                            out=xkh,
                            in_=xkTb[
                                h * 512 : (h + 1) * 512, i * 512 : (i + 1) * 512
                            ].rearrange("(c p) q -> p c q", p=128),
                        )
                        xk_t[h][i] = xkh

            # ---- merged loop: projections(i) + attention(qc=i) ----
            for i in range(NQC):
                qsl = slice(i * 512, (i + 1) * 512)
                xq_h = [xq_t[0][i], xq_t[1][i]]

                # QT: qt[d][:, qsl] = (x Wq)^T
                for d in range(2):
                    dsl = slice(d * 128, (d + 1) * 128)
                    ps = psA.tile([128, 512], F32, tag="psA")
                    for c in range(8):
                        nc.tensor.matmul(
                            ps,
                            lhsT=wq_b[c][:, dsl],
                            rhs=xq_h[c // 4][:, c % 4, :],
                            start=(c == 0),
                            stop=(c == 7),
                        )
                    nc.vector.tensor_copy(qt[d][:, qsl], ps)

                if i < NKC:
                    xk_h = [xk_t[0][i], xk_t[1][i]]
                    ksl = slice(i * 512, (i + 1) * 512)
                    # KT: kt[d][:, ksl] = (xk Wk)^T
                    for d in range(2):
                        dsl = slice(d * 128, (d + 1) * 128)
                        psk = psB.tile([128, 512], F32, tag="psB")
                        for c in range(8):
                            nc.tensor.matmul(
                                psk,
                                lhsT=wk_b[c][:, dsl],
                                rhs=xk_h[c // 4][:, c % 4, :],
                                start=(c == 0),
                                stop=(c == 7),
                            )
                        nc.vector.tensor_copy(kt[d][:, ksl], psk)
                    # Vd[kb] = xk_blk Wvd, [k, d] layout (x-block stationary)
                    for kb in range(4 * i, 4 * i + 4):
                        p4 = kb % 4
                        psv = psB.tile([128, 256], F32, tag="psB")
                        for c in range(8):
                            nc.tensor.matmul(
                                psv,
                                lhsT=xk_h[c // 4][:, c % 4, p4 * 128 : (p4 + 1) * 128],
                                rhs=wvd_b[c],
                                start=(c == 0),
                                stop=(c == 7),
                            )
                        nc.vector.tensor_copy(vd[kb], psv)

                # ---------------- attention for qc = i ----------------
                nb = 2 * i + 2
                pts = []
                for kb in range(nb):
                    st = psA.tile([128, 512], F32, tag="psA")
                    for d in range(2):
                        nc.tensor.matmul(
                            st,
                            lhsT=kt[d][:, kb * 128 : (kb + 1) * 128],
                            rhs=qt[d][:, qsl],
                            start=(d == 0),
                            stop=(d == 1),
                        )
                    pt = ppool.tile([128, 512], BF16, tag=f"p{kb}", name=f"p{kb}")
                    nc.scalar.activation(
                        pt, st, mybir.ActivationFunctionType.Exp, scale=float(SCALE)
                    )
                    # The last two blocks straddle the causal diagonal.
                    if kb == nb - 2:
                        nc.vector.tensor_mul(pt, pt, mask_a)
                    elif kb == nb - 1:
                        nc.vector.tensor_mul(pt, pt, mask_b)
                    pts.append(pt)

                # softmax denominator prep: group P tiles by 4 on vector;
                # these adds overlap the O'T matmul burst below.
                groups = [pts[j : j + 4] for j in range(0, nb, 4)]
                grhs = []
                for gi, g in enumerate(groups):
                    if len(g) == 1:
                        grhs.append(g[0])
                    else:
                        acc = ppool.tile([128, 512], BF16, tag=f"s4_{gi}", name=f"s4_{gi}")
                        nc.vector.tensor_add(acc, g[0], g[1])
                        for t in g[2:]:
                            nc.vector.tensor_add(acc, acc, t)
                        grhs.append(acc)

                # O'^T[d, q] += Vd[kb]^T P[kb]   (Vd blocks stationary)
                ot = [
                    psOT.tile([128, 512], F32, tag=f"ot{d2}", name=f"ot{d2}")
                    for d2 in range(2)
                ]
                for kb in range(nb):
                    for d2 in range(2):
                        nc.tensor.matmul(
                            ot[d2],
                            lhsT=vd[kb][:, d2 * 128 : (d2 + 1) * 128],
                            rhs=pts[kb],
                            start=(kb == 0),
                            stop=(kb == nb - 1),
                        )
                ot_sb = []
                for d2 in range(2):
                    t = stage.tile([128, 512], BF16, tag=f"otsb{d2}", bufs=2)
                    nc.vector.tensor_copy(t, ot[d2])
                    ot_sb.append(t)

                # sums[1, q] += 1^T P4[k, q]  (inputs ready: adds overlapped)
                sums = psS.tile([1, 512], F32, tag="sums")
                for gi, g in enumerate(grhs):
                    nc.tensor.matmul(
                        sums,
                        lhsT=ones,
                        rhs=g,
                        start=(gi == 0),
                        stop=(gi == len(grhs) - 1),
                    )
                ssb = stage.tile([1, 512], F32, tag="ssb")
                nc.vector.tensor_copy(ssb, sums)
                nc.gpsimd.dma_start(out=ssum[i : i + 1, :], in_=ssb)

                # out[q, e] = O'[q, d] Wvu[d, e]  (O'^T blocks stationary)
                for qb in range(4):
                    qbsl = slice(qb * 128, (qb + 1) * 128)
                    for eh in range(2):
                        esl = slice(eh * 512, (eh + 1) * 512)
                        po = psA.tile([128, 512], F32, tag="psA")
                        for d2 in range(2):
                            nc.tensor.matmul(
                                po,
                                lhsT=ot_sb[d2][:, qbsl],
                                rhs=wvu_b[d2][:, esl],
                                start=(d2 == 0),
                                stop=(d2 == 1),
                            )
                        ob = stage.tile([128, 512], BF16, tag=f"ob{eh}", bufs=4, name=f"ob{eh}")
                        if eh == 0:
                            nc.vector.tensor_copy(ob, po)
                        else:
                            nc.scalar.activation(
                                ob, po, mybir.ActivationFunctionType.Copy
                            )
                        nc.sync.dma_start(
                            out=o[(i * 4 + qb) * 128 : (i * 4 + qb + 1) * 128, esl],
                            in_=ob,
                        )
    nc.finalize()
    return nc


def _get_nc():
    if "nc" not in _CACHE:
        _CACHE["nc"] = _build_nc()
    return _CACHE["nc"]


def _host_masks(parity: int):
    import ml_dtypes

    y = np.arange(512)[None, :]
    x = np.arange(128)[:, None]
    mask_a = (y - x - 128 * parity >= 0).astype(ml_dtypes.bfloat16)
    mask_b = (y - x - 256 - 128 * parity >= 0).astype(ml_dtypes.bfloat16)
    return mask_a, mask_b


def _make_in_maps(inputs, Wq, Wk, Wvdown, Wvup):
    import ml_dtypes

    bf16 = ml_dtypes.bfloat16
    in_maps = []
    wq_b = np.ascontiguousarray(np.asarray(Wq, np.float32).astype(bf16))
    wk_b = np.ascontiguousarray(np.asarray(Wk, np.float32).astype(bf16))
    wvd_b = np.ascontiguousarray(np.asarray(Wvdown, np.float32).astype(bf16))
    wvu_b = np.ascontiguousarray(np.asarray(Wvup, np.float32).astype(bf16))
    xb16 = [np.asarray(inputs[b], np.float32).astype(bf16) for b in range(B)]
    xTs = [np.ascontiguousarray(xb16[b].T) for b in range(B)]
    for core in range(NCORES):
        b, parity = core // 2, core % 2
        xk = np.ascontiguousarray(
            xb16[b].reshape(N // 128, 128, E)[parity::2].reshape(KLOC, E)
        )
        xkT = np.ascontiguousarray(xk.T)  # [E, KLOC] bf16
        mask_a, mask_b = _host_masks(parity)
        in_maps.append(
            {
                "xTb": xTs[b],
                "xkTb": xkT,
                "wq": wq_b,
                "wk": wk_b,
                "wvd": wvd_b,
                "wvu": wvu_b,
                "mka": mask_a,
                "mkb": mask_b,
            }
        )
    return in_maps


def _combine(results):
    out = np.empty((B, N, E), dtype=np.float32)
    for b in range(B):
        o_sum = results[2 * b]["o"].astype(np.float32) + results[2 * b + 1][
            "o"
        ].astype(np.float32)
        s_sum = (results[2 * b]["ssum"] + results[2 * b + 1]["ssum"]).reshape(N)
        out[b] = o_sum / s_sum[:, None]
    return out


def kernel(inputs, Wq, Wk, Wvdown, Wvup):
    from concourse.bass_utils import run_bass_kernel_spmd

    inputs = np.asarray(inputs, dtype=np.float32)
    in_maps = _make_in_maps(inputs, Wq, Wk, Wvdown, Wvup)
    res = run_bass_kernel_spmd(_get_nc(), in_maps, core_ids=list(range(NCORES)))
    return _combine(res.results)
